# revision 18
# baseline (speedup 1.0000x reference)
"""Trainium2 Bass kernel for nn_MultiHeadAttention_81673098101666.

Reference computation (per batch b):
    qkv  = seq @ w_qkv.T ; q,k,v = split(qkv)        # seq [S,128], q/k/v [S,1024]
    scores = q @ k.T / 32 ; attn = softmax(scores)
    out  = attn @ v @ w_out.T + b_out                # [S, 128]

Key algebraic identity (INPUT_DIM=128 => rank-128 attention):
    scoresT = A^T-contracted against seq_q   with A = M^T seqT, M = Wk^T Wq
    outT    = W2T^T (seqT E^T) / sumexp      with W2T = Wv^T Wout^T
so the S^2-sized matmuls contract over 128 dims instead of 1024 and Q/K/V
are never materialized.  A, W2T, the 1/sumexp division and the bias are all
folded on the HOST (A is a [2048,128]@[128,128] per batch - cheap), so the
device does only: scores matmuls, exp, the C = seqT E^T accumulation, and
bf16 partial-sums of E for the softmax denominator.

Sharding: 8 cores = 4 batches x 2 query-halves; no collectives.

Performance design (per core):
  - hard floor: the exp chain on the scalar engine - 2M elements at
    1 elem/cycle/lane = ~16.2us, strictly serial.  Everything else (PE
    matmuls ~14us, DVE adds, DMA) overlaps underneath it.
  - every DMA transfer pays ~1.5us of issue->first-byte plus ~0.8us
    last-byte->completion-semaphore latency, and all rings share the 16
    SDMA engines round-robin.  So the chain-critical first transfer (AT
    kt0 + first query chunk) is SPLIT BY PARTITION HALVES across both HW
    rings (80KB each), the second chunk rides the SW ring's first slot,
    and all bulk transfers sit in second/third slots with >=2us slack.
  - first/last key-tiles' exp split into [128,512] halves: EXP0a needs
    only the first transfer; EXP15a lets the final C matmuls + copies
    begin before the chain's last ACT retires.
  - warm-up matmuls (memset on gpsimd at high priority, the earliest
    writer) keep PE busy from ~7.3us so the HAM clock gate (1.2 -> 2.4
    GHz) releases before the scores stream; dependency-free FILLER
    matmuls into the still-idle C banks hold PE utilization through the
    chain ramp-up so the HAM does not down-clock mid-chain (it samples
    utilization on ~3.4us windows); C matmuls lag the scores by two kt.
  - sumexp: exp tiles for kt1..14 are written into [128,2048] PAIRS; the
    DVE folds pairs of et0..12 into accF early (its 256KB DMA hides
    under the chain on the gpsimd ring); et13+et14+et15 fold into a
    second partial accG with one wide add + one trailing add.  The host
    does the 128-row reduction of both partials.
  - tail: C0 copy+DMA on scalar (idle after the chain), C1 copy on
    vector with DMA on sync, accG on the gpsimd ring - three rings in
    parallel, each output issued as soon as its producer retires.
"""

import numpy as np

B, S, DIN = 4, 2048, 128
O = 1024
QPC = S // 2           # queries per core = 1024
QC = 512               # query-chunk width (PSUM bank limit: 512 fp32)
NKT = S // 128         # 16 key tiles
SCALE = 1.0 / 32.0     # 1/sqrt(O)

_NC = None
PROFILE = False
LAST_RESULTS = None


def _body(ctx, tc, ins, outT_d, accf_d, accg_d):
    import concourse.mybir as mybir

    nc = tc.nc
    f32 = mybir.dt.float32
    b16 = mybir.dt.bfloat16
    i16 = mybir.dt.int16
    Exp = mybir.ActivationFunctionType.Exp
    add = mybir.AluOpType.add
    mult = mybir.AluOpType.mult

    consts = ctx.enter_context(tc.tile_pool(name="consts", bufs=1))
    et_pool = ctx.enter_context(tc.tile_pool(name="et", bufs=9))
    acc_pool = ctx.enter_context(tc.tile_pool(name="accp", bufs=3))
    c_pool = ctx.enter_context(tc.tile_pool(name="cp", bufs=2))
    psum = ctx.enter_context(tc.tile_pool(name="psum", bufs=1, space="PSUM"))

    # ---- SBUF tiles: one per input transfer (HQ0 split by partitions) --
    HQ0 = consts.tile([128, 640], b16)    # [AT kt0 | queries 0:512]
    HQ1 = consts.tile([128, 640], b16)    # [AT kt1 | queries 512:1024]
    GA = consts.tile([128, 768], b16)     # AT kt2..7
    SC2 = consts.tile([128, 512], b16)    # seqn kt0..3
    SY2 = consts.tile([128, 1536], b16)   # [AT kt8..15 | seqn kt4..7]
    GC = consts.tile([128, 1024], b16)    # seqn kt8..15
    warm_sb = consts.tile([128, QC], b16)

    def at_sl(kt):
        if kt == 0:
            return HQ0[:, 0:128]
        if kt == 1:
            return HQ1[:, 0:128]
        if kt < 8:
            return GA[:, (kt - 2) * 128:(kt - 1) * 128]
        return SY2[:, (kt - 8) * 128:(kt - 7) * 128]

    def sn_sl(kt):
        if kt < 4:
            return SC2[:, kt * 128:(kt + 1) * 128]
        if kt < 8:
            return SY2[:, 1024 + (kt - 4) * 128:1024 + (kt - 3) * 128]
        return GC[:, (kt - 8) * 128:(kt - 7) * 128]

    qrhs = [HQ0[:, 128:640], HQ1[:, 128:640]]

    # ---- warm-up init on scalar (boots early, ahead of its DMA issues),
    # so PE warm-ups start ~7.3us and the HAM clock ramp (needs ~2.7us of
    # sustained PE activity) completes BEFORE the exp chain begins --------
    with tc.high_priority():
        nc.gpsimd.memset(warm_sb[:], 1.0)

    # ---- input DMAs: chain-critical first on every ring ----------------
    nc.sync.dma_start(HQ0[:], ins["h0"])
    nc.scalar.dma_start(HQ1[:], ins["h1"])
    nc.gpsimd.dma_start(GA[:], ins["ga"])
    nc.sync.dma_start(SY2[:], ins["sy2"])
    nc.scalar.dma_start(SC2[:], ins["sc2"])
    nc.gpsimd.dma_start(GC[:], ins["gc"])

    # warm-up matmuls: keep PE busy through the DMA head so the HAM
    # clock-gate releases (1.2 -> 2.4 GHz) before the real matmul stream
    for w in range(7):
        wid = QC if w < 5 else 256
        pw = psum.tile([128, QC], f32, tag="mm", bufs=3, name=f"pw{w}")
        nc.tensor.matmul(pw[:, 0:wid], warm_sb[:, 0:128], warm_sb[:, 0:wid],
                         start=True, stop=True)

    # ---- C accumulation banks ------------------------------------------
    pcs = [psum.tile([128, QC], f32, tag="ctx", bufs=2, name=f"pc{qc}")
           for qc in range(2)]

    def score_half(kt, qc, et_dst, name):
        # single [128,512] scores matmul + exp (first / last key tile)
        pp = psum.tile([128, QC], f32, tag="mm", bufs=3, name=name)
        nc.tensor.matmul(pp[:], at_sl(kt), qrhs[qc], start=True, stop=True)
        nc.scalar.activation(et_dst, pp[:], Exp, scale=float(SCALE))

    # Schraudolph bf16 exp for DVE-offloaded tiles: exp(x*SCALE) ~=
    # bitcast_bf16(int16(x * 128/(ln2*32) + 16250.6)); ~3.3% max rel err,
    # cancels in the softmax ratio.  One fused tensor_scalar per tile
    # relieves the scalar-engine ACT chain (the kernel's critical path).
    SCH_A = float(128.0 / (np.log(2.0) * 32.0))
    SCH_B = 16250.6

    def score_tile(kt, et_dst, dve=False):
        pp = psum.tile([128, 1024], f32, tag="mm", bufs=3, name=f"pp{kt}")
        for qc in range(2):
            nc.tensor.matmul(pp[:, qc * QC:(qc + 1) * QC], at_sl(kt),
                             qrhs[qc], start=True, stop=True,
                             skip_group_check=True)
        if dve:
            nc.vector.tensor_scalar(et_dst.bitcast(i16), pp[:],
                                    SCH_A, SCH_B, mult, add)
        else:
            nc.scalar.activation(et_dst, pp[:], Exp, scale=float(SCALE))

    def c_mm(kt, et_sl, first=False, last=False):
        for qc in range(2):
            nc.tensor.matmul(pcs[qc][:], sn_sl(kt),
                             et_sl[:, qc * QC:(qc + 1) * QC],
                             start=first, stop=last)

    # et storage: kt0 / kt15 as standalone [128,1024]; kt1..14 as pairs
    et0 = et_pool.tile([128, 1024], b16, tag="et", name="et0")
    et15 = et_pool.tile([128, 1024], b16, tag="et", name="et15")
    prs = [et_pool.tile([128, 2048], b16, tag="et", name=f"etp{p}")
           for p in range(7)]
    esl = {0: et0, 15: et15}
    for kt in range(1, 15):
        p, half = (kt - 1) // 2, (kt - 1) % 2
        esl[kt] = prs[p][:, half * 1024:(half + 1) * 1024]

    accP = acc_pool.tile([128, 2048], b16, tag="acc", name="accP")
    accF = acc_pool.tile([128, 1024], b16, tag="acc", name="accF")
    accG = acc_pool.tile([128, 1024], b16, tag="acc", name="accG")

    def pe_filler(n):
        # dependency-free matmuls into the (still idle) C banks: they keep
        # PE utilization high through the chain ramp-up so the HAM does
        # not down-clock mid-chain; the first real C matmul's start=True
        # resets the bank, so the garbage is never observed
        for i in range(n):
            nc.tensor.matmul(pcs[i % 2][:], warm_sb[:, 0:128], warm_sb[:],
                             start=True, stop=True)

    # ---- main stream: scores/exp lead, C matmuls lag one kt ------------
    score_half(0, 0, et0[:, 0:QC], "pp0a")
    pe_filler(3)
    score_half(0, 1, et0[:, QC:1024], "pp0b")
    pe_filler(3)
    for kt in range(1, 15):
        score_tile(kt, esl[kt])
        if kt in (1, 2):
            pe_filler(3)
        if kt >= 2:
            c_mm(kt - 2, esl[kt - 2], first=(kt == 2))
        # DVE pair-accumulation, woven in as pairs complete (covers et0..12)
        if kt == 5:
            nc.vector.tensor_tensor(accP[:], prs[0][:], prs[1][:], add)
            nc.vector.tensor_tensor(accP[:, 0:1024], accP[:, 0:1024],
                                    et0[:], add)
        elif kt in (7, 9, 11, 13):
            nc.vector.tensor_tensor(accP[:], accP[:], prs[(kt - 3) // 2][:],
                                    add)

    # accF = et0..12: fold and ship it while the chain is still running
    nc.vector.tensor_tensor(accF[:], accP[:, 0:1024], accP[:, 1024:2048], add)
    nc.gpsimd.dma_start(accf_d[:], accF[:])

    score_half(15, 0, et15[:, 0:QC], "pp15a")
    c_mm(13, esl[13])
    # accG = et13 + et14 (prs[6] halves), ready right after ACT14
    nc.vector.tensor_tensor(accG[:], prs[6][:, 0:1024], prs[6][:, 1024:2048],
                            add)
    score_half(15, 1, et15[:, QC:1024], "pp15b")
    c_mm(14, esl[14])
    nc.tensor.matmul(pcs[0][:], sn_sl(15), et15[:, 0:QC],
                     start=False, stop=True)
    nc.tensor.matmul(pcs[1][:], sn_sl(15), et15[:, QC:1024],
                     start=False, stop=True)

    # trailing partial: accG += et15, then ship on the gpsimd ring
    nc.vector.tensor_tensor(accG[:], accG[:], et15[:], add)
    nc.gpsimd.dma_start(accg_d[:], accG[:])

    # ---- outputs: both C copies on scalar (idle post-chain), both DMA
    # issues on sync, so the vector tail is only the accG adds -----------
    C0_sb = c_pool.tile([128, QC], b16, tag="c", name="C0")
    nc.scalar.copy(C0_sb[:], pcs[0][:])
    nc.sync.dma_start(outT_d[:, 0:QC], C0_sb[:])

    C1_sb = c_pool.tile([128, QC], b16, tag="c", name="C1")
    nc.scalar.copy(C1_sb[:], pcs[1][:])
    nc.sync.dma_start(outT_d[:, QC:2 * QC], C1_sb[:])


def _build_nc():
    from contextlib import ExitStack

    import concourse.mybir as mybir
    import concourse.tile as tile
    from concourse import bacc

    b16 = mybir.dt.bfloat16
    nc = bacc.Bacc("TRN2", target_bir_lowering=False, debug=False, num_devices=8)
    shapes = {
        "h0": [128, 640], "h1": [128, 640],
        "ga": [128, 768], "sc2": [128, 512], "sy2": [128, 1536],
        "gc": [128, 1024],
    }
    ins = {k: nc.dram_tensor(k, sh, b16, kind="ExternalInput").ap()
           for k, sh in shapes.items()}
    outT_d = nc.dram_tensor("outT", [128, QPC], b16, kind="ExternalOutput").ap()
    accf_d = nc.dram_tensor("accf", [128, QPC], b16, kind="ExternalOutput").ap()
    accg_d = nc.dram_tensor("accg", [128, QPC], b16, kind="ExternalOutput").ap()

    with tile.TileContext(nc) as tc:
        with ExitStack() as ctx:
            _body(ctx, tc, ins, outT_d, accf_d, accg_d)
    nc.compile()
    return nc


def get_nc():
    global _NC
    if _NC is None:
        _NC = _build_nc()
    return _NC


def make_in_maps(sequence, w_qkv):
    import ml_dtypes

    bf16 = ml_dtypes.bfloat16
    wq, wk = w_qkv[:O], w_qkv[O:2 * O]
    M = wk.T @ wq                                     # [128, 128]

    in_maps = []
    for b in range(B):
        seq = sequence[b]                             # [2048, 128] fp32
        AT = np.ascontiguousarray((seq @ M).T.astype(bf16))   # [128, 2048]
        seq16 = seq.astype(bf16)
        seqT = np.ascontiguousarray(seq16.T)          # [128, 2048]
        # seqn tiled: partition p holds [t, i] for key t*128+p
        seqn = np.ascontiguousarray(
            seq16.reshape(NKT, 128, 128).transpose(1, 0, 2).reshape(128, S))
        ga = np.ascontiguousarray(AT[:, 256:1024])
        sc2 = np.ascontiguousarray(seqn[:, 0:512])
        sy2 = np.ascontiguousarray(
            np.concatenate([AT[:, 1024:2048], seqn[:, 512:1024]], axis=1))
        gc = np.ascontiguousarray(seqn[:, 1024:2048])
        for h in range(2):
            q = seqT[:, h * QPC:(h + 1) * QPC]
            in_maps.append({
                "h0": np.ascontiguousarray(
                    np.concatenate([AT[:, 0:128], q[:, 0:QC]], axis=1)),
                "h1": np.ascontiguousarray(
                    np.concatenate([AT[:, 128:256], q[:, QC:QPC]], axis=1)),
                "ga": ga, "sc2": sc2, "sy2": sy2, "gc": gc,
            })
    return in_maps


def kernel(sequence, w_qkv, w_out, b_out):
    global LAST_RESULTS
    from concourse.bass_utils import run_bass_kernel_spmd

    sequence = np.asarray(sequence, dtype=np.float32)
    w_qkv = np.asarray(w_qkv, dtype=np.float32)
    w_out = np.asarray(w_out, dtype=np.float32)
    b_out = np.asarray(b_out, dtype=np.float32)

    nc = get_nc()
    in_maps = make_in_maps(sequence, w_qkv)
    kw = {}
    if PROFILE:
        kw = dict(trace=True, trace_cores=[0])
    res = run_bass_kernel_spmd(nc, in_maps, list(range(8)), **kw)
    LAST_RESULTS = res

    wv = w_qkv[2 * O:]
    W2T = (wv.T @ w_out.T).astype(np.float32)              # [128, 128]
    out = np.empty((B, S, DIN), np.float32)
    for c in range(8):
        b, h = c // 2, c % 2
        C = res.results[c]["outT"].astype(np.float32)      # [128,1024] seqT E^T
        se = (res.results[c]["accf"].astype(np.float32).sum(axis=0)
              + res.results[c]["accg"].astype(np.float32).sum(axis=0))
        outT = W2T.T @ C                                   # [128, 1024]
        out[b, h * QPC:(h + 1) * QPC, :] = outT.T / se[:, None] + b_out[None, :]
    return out
